# revision 11
# baseline (speedup 1.0000x reference)
"""Trainium2 Bass kernel for nn_NCC2D_44856638439947.

Computes -mean(cc) where cc is the local (9x9 window) normalized
cross-correlation between y_true and y_pred, B=32, H=W=512, fp32.

Strategy (8 NeuronCores, pure data parallel over batch):
  - each core gets 4 images; host sums the 8 per-core partial sums.
  - per image, 5 product maps (I, J, I*J, I^2, J^2) are box-filtered 9x9
    via two TensorE matmul passes against banded all-ones matrices:
      pass 1 (data-stationary): T1 = X^T @ A1   -> [x, y'] orientation
      pass 2 (band-stationary): M  = A2^T @ T1  -> [x', y'] orientation
    The orientation flip in pass 1 makes the second contraction (over x)
    a partition-dim contraction, so no transposes are ever needed.
  - final per-pixel cc math runs on VectorE/ScalarE in fp16 with the
    1/81 normalizations folded into the A2 weights (cc is invariant to
    a uniform scale alpha on a,b and alpha^2 on C,D,E).
  - per-core output: [128,1] fp32 partial sums of cc; host reduces.
"""

import numpy as np

H = W = 512
WIN = 9
PAD = WIN // 2
B = 32
NCORES = 8
IMGS = B // NCORES  # 4 images per core

# per-pass box weight 1.0; per-line normalization folded into A2:
#   a' = a/16, b' = b/16  (alpha = 1/16)
#   C' = 81c/256, D' = 81d/256, E' = 81e/256  (alpha^2 * 81)
# => cc = (E'-a'b')^2 / ((C'-a'^2)(D'-b'^2) + eps*81^2*alpha^4), exactly
#    equal to the reference cc = cross^2/(I_var*J_var + 1e-5).
W_A2_LINE = 1.0 / 16.0  # weight for a,b lines (applied in pass 2)
W_A2_PROD = 81.0 / 256.0  # weight for C,D,E lines (applied in pass 2)
EPS_T = (81.0 * 81.0 * 1e-5) / (16.0**4)

# pass-1 y' output windows per y-block b (start, stop)
Y1WIN = [(0, 132), (124, 260), (252, 388), (380, 512)]
# pass-2 x windows: (x_start, x_count, xp_start, xp_count, variant)
#   variant 0: band(r - c)      (w == 0, left edge)
#   variant 1: band(r - 4 - c)  (interior and right edge)
XWIN = [
    (0, 128, 0, 120, 0),
    (116, 128, 120, 120, 1),
    (236, 128, 240, 120, 1),
    (356, 128, 360, 120, 1),
    (476, 36, 480, 32, 1),
]
NSETS = IMGS * len(XWIN)  # accumulator columns per core


def _band_matrices():
    """Build the fp16 banded matrices shipped to the device."""
    # A1: concat over b of [128, n_b]; A1_b[k, j] = 1 if |128b+k - (ys+j)| <= 4
    a1_parts = []
    for bb, (ys, ye) in enumerate(Y1WIN):
        k = np.arange(128)[:, None] + 128 * bb
        j = np.arange(ys, ye)[None, :]
        a1_parts.append((np.abs(k - j) <= PAD).astype(np.float16))
    a1 = np.concatenate(a1_parts, axis=1)  # [128, 536]

    # A2 variants, shape [128, 120] / [128, 120] / padded [128, 32]
    def band(rows, cols, shift):
        r = np.arange(rows)[:, None]
        c = np.arange(cols)[None, :]
        return (np.abs(r - shift - c) <= PAD).astype(np.float16)

    v0 = band(128, 120, 0)
    v1 = band(128, 120, 4)
    v4 = np.zeros((128, 32), np.float16)
    v4[:36] = band(36, 32, 4)
    a2 = np.concatenate([v0, v1, v4], axis=1)  # [128, 272]
    return a1, (a2 * np.float16(W_A2_LINE)).astype(np.float16), (
        a2 * np.float16(W_A2_PROD)
    ).astype(np.float16)


A2OFF = {0: 0, 1: 120, 4: 240}  # column offset of each variant in A2 arrays


def build_bass(reps=1):
    """Build the Bass program (SPMD, identical on all cores)."""
    from contextlib import ExitStack

    import concourse.tile as tile
    from concourse import bacc, mybir

    f32 = mybir.dt.float32
    f16 = mybir.dt.float16
    Act = mybir.ActivationFunctionType
    Alu = mybir.AluOpType

    nc = bacc.Bacc(
        "TRN2",
        target_bir_lowering=False,
        debug=False,
        num_devices=NCORES,
    )
    yt = nc.dram_tensor("y_true", [IMGS, H, W], f32, kind="ExternalInput").ap()
    yp = nc.dram_tensor("y_pred", [IMGS, H, W], f32, kind="ExternalInput").ap()
    a1_d = nc.dram_tensor("A1", [128, 536], f16, kind="ExternalInput").ap()
    a2w_d = nc.dram_tensor("A2W", [128, 272], f16, kind="ExternalInput").ap()
    a2b_d = nc.dram_tensor("A2B", [128, 272], f16, kind="ExternalInput").ap()
    out_d = nc.dram_tensor("partial", [128, 1], f32, kind="ExternalOutput").ap()

    with tile.TileContext(nc) as tc, ExitStack() as ctx:
        consts = ctx.enter_context(tc.tile_pool(name="consts", bufs=1))
        in_pool = ctx.enter_context(tc.tile_pool(name="inp", bufs=2))
        map_pool = ctx.enter_context(tc.tile_pool(name="maps", bufs=2))
        t1_pool = ctx.enter_context(tc.tile_pool(name="t1", bufs=2))
        fin_pool = ctx.enter_context(tc.tile_pool(name="fin", bufs=2))
        acc_pool = ctx.enter_context(tc.tile_pool(name="acc", bufs=1))
        ps1 = ctx.enter_context(tc.tile_pool(name="ps1", bufs=2, space="PSUM"))
        ps2 = ctx.enter_context(tc.tile_pool(name="ps2", bufs=1, space="PSUM"))

        a1 = consts.tile([128, 536], f16)
        nc.sync.dma_start(a1[:], a1_d[:])
        a2w = consts.tile([128, 272], f16)
        nc.sync.dma_start(a2w[:], a2w_d[:])
        a2b = consts.tile([128, 272], f16)
        nc.sync.dma_start(a2b[:], a2b_d[:])

        acc = acc_pool.tile([128, NSETS], f32)
        nc.gpsimd.memset(acc[:], 0.0)
        eps_c = consts.tile([128, 1], f32)
        nc.gpsimd.memset(eps_c[:], EPS_T)

        # A1 column offsets per y-block
        a1off = [0, 132, 268, 404]

        drain_flip = 0
        for img in range(IMGS * reps):
            img = img % IMGS
            iblk = []
            jblk = []
            for bb in range(4):
                it = in_pool.tile([128, W], f32, tag=f"i{bb}")
                nc.sync.dma_start(it[:], yt[img, 128 * bb : 128 * (bb + 1), :])
                jt = in_pool.tile([128, W], f32, tag=f"j{bb}")
                nc.sync.dma_start(jt[:], yp[img, 128 * bb : 128 * (bb + 1), :])
                iblk.append(it)
                jblk.append(jt)

            # pre: 5 fp16 product maps, each 4 blocks of [128, 512]
            maps = []  # maps[m][b]
            for m in range(5):
                maps.append([])
            for bb in range(4):
                mi = map_pool.tile([128, W], f16, tag=f"mI{bb}")
                nc.vector.tensor_copy(mi[:], iblk[bb][:])
                mj = map_pool.tile([128, W], f16, tag=f"mJ{bb}")
                nc.vector.tensor_copy(mj[:], jblk[bb][:])
                mij = map_pool.tile([128, W], f16, tag=f"mIJ{bb}")
                nc.vector.tensor_tensor(
                    mij[:], iblk[bb][:], jblk[bb][:], op=Alu.mult
                )
                mi2 = map_pool.tile([128, W], f16, tag=f"mI2{bb}")
                nc.scalar.activation(mi2[:], iblk[bb][:], Act.Square)
                mj2 = map_pool.tile([128, W], f16, tag=f"mJ2{bb}")
                nc.scalar.activation(mj2[:], jblk[bb][:], Act.Square)
                for m, t in enumerate([mi, mj, mij, mi2, mj2]):
                    maps[m].append(t)

            # pass 1 + drain: T1[m][w] fp16 [128 (x win), 512 (y')]
            t1 = [[None] * 5 for _ in range(5)]
            for m in range(5):
                for w, (xs, xc, _, _, _) in enumerate(XWIN):
                    p1 = ps1.tile([128, W], f32, tag="p1")
                    for bb in range(4):
                        ys, ye = Y1WIN[bb]
                        lhs = maps[m][bb][:, xs : xs + xc]
                        if bb == 0:
                            nc.tensor.matmul(
                                p1[0:xc, ys:ye],
                                lhsT=lhs,
                                rhs=a1[:, a1off[bb] : a1off[bb] + (ye - ys)],
                                start=True,
                                stop=False,
                            )
                        else:
                            # split: 8-col overlap (accumulates) + fresh cols,
                            # so each instruction touches a uniform PSUM region
                            mid = ys + 8
                            nc.tensor.matmul(
                                p1[0:xc, ys:mid],
                                lhsT=lhs,
                                rhs=a1[:, a1off[bb] : a1off[bb] + 8],
                                start=False,
                                stop=False,
                            )
                            nc.tensor.matmul(
                                p1[0:xc, mid:ye],
                                lhsT=lhs,
                                rhs=a1[:, a1off[bb] + 8 : a1off[bb] + (ye - ys)],
                                start=False,
                                stop=(bb == 3),
                            )
                    t = t1_pool.tile([128, W], f16, tag=f"t1_{m}_{w}")
                    if drain_flip % 2 == 0:
                        nc.vector.tensor_copy(t[0:xc, :], p1[0:xc, :])
                    else:
                        nc.scalar.activation(t[0:xc, :], p1[0:xc, :], Act.Copy)
                    drain_flip += 1
                    t1[m][w] = t

            # pass 2 + final, per x window
            for w, (xs, xc, xps, xpc, var) in enumerate(XWIN):
                o = A2OFF[4 if w == 4 else var]
                p2 = []
                for m in range(5):
                    a2 = a2w if m < 2 else a2b
                    pt = ps2.tile([128, W], f32, tag=f"p2_{m}")
                    nc.tensor.matmul(
                        pt[0:xpc, :],
                        lhsT=a2[0:xc, o : o + xpc],
                        rhs=t1[m][w][0:xc, :],
                        start=True,
                        stop=True,
                    )
                    p2.append(pt)
                pa, pb, pe, pc, pd = (t[0:xpc, :] for t in p2)

                def ftile(tag):
                    t = fin_pool.tile([128, W], f16, tag=tag, name=tag)
                    return t[0:xpc, :]

                a_s = ftile("a_s")
                nc.scalar.activation(a_s, pa, Act.Copy)
                ab = ftile("ab")
                nc.vector.tensor_tensor(ab, a_s, pb, op=Alu.mult)
                a2s = ftile("a2s")
                nc.vector.tensor_tensor(a2s, a_s, a_s, op=Alu.mult)
                b2s = ftile("b2s")
                nc.scalar.activation(b2s, pb, Act.Square)
                cross = ftile("cross")
                nc.vector.tensor_tensor(cross, pe, ab, op=Alu.subtract)
                iv = ftile("iv")
                nc.vector.tensor_tensor(iv, pc, a2s, op=Alu.subtract)
                jv = ftile("jv")
                nc.vector.tensor_tensor(jv, pd, b2s, op=Alu.subtract)
                den = ftile("den")
                nc.vector.tensor_tensor(den, iv, jv, op=Alu.mult)
                lg = ftile("lg")
                nc.scalar.activation(lg, den, Act.Ln, bias=eps_c[0:xpc, :])
                rr = ftile("rr")
                nc.scalar.activation(rr, lg, Act.Exp, scale=-0.5)
                q = ftile("q")
                nc.vector.tensor_tensor(q, cross, rr, op=Alu.mult)
                scr = ftile("scr")
                col = img * 5 + w
                nc.vector.scalar_tensor_tensor(
                    scr,
                    q,
                    1.0,
                    q,
                    op0=Alu.mult,
                    op1=Alu.mult,
                    accum_out=acc[0:xpc, col : col + 1],
                )

        red = acc_pool.tile([128, 1], f32)
        nc.vector.tensor_reduce(
            red[:], acc[:, 0:NSETS], axis=mybir.AxisListType.X, op=Alu.add
        )
        nc.sync.dma_start(out_d[:], red[:])

    nc.compile()
    return nc


_CACHED = {}


def kernel(y_true: np.ndarray, y_pred: np.ndarray) -> np.ndarray:
    from concourse import bass_utils

    if "nc" not in _CACHED:
        _CACHED["nc"] = build_bass()
    nc = _CACHED["nc"]

    a1, a2w, a2b = _band_matrices()
    yt = np.ascontiguousarray(y_true.reshape(B, H, W), dtype=np.float32)
    yp = np.ascontiguousarray(y_pred.reshape(B, H, W), dtype=np.float32)
    in_maps = []
    for c in range(NCORES):
        in_maps.append(
            {
                "y_true": yt[IMGS * c : IMGS * (c + 1)],
                "y_pred": yp[IMGS * c : IMGS * (c + 1)],
                "A1": a1,
                "A2W": a2w,
                "A2B": a2b,
            }
        )
    res = bass_utils.run_bass_kernel_spmd(
        nc, in_maps, core_ids=list(range(NCORES))
    )
    total = 0.0
    for c in range(NCORES):
        total += float(res.results[c]["partial"].astype(np.float64).sum())
    mean = total / float(B * H * W)
    return np.float32(-mean)
